# revision 6
# baseline (speedup 1.0000x reference)
"""AttentionBlock (GroupNorm + 1x1-conv QKV self-attention + residual) on 8 TRN2 cores.

Sharding: data-parallel over batch B=4 x sequence-parallel over the 4096
tokens (2 cores per batch element, each handling 2048 query rows; K/V and
GroupNorm are computed redundantly per core pair — they are cheap relative
to attention).

Per-core device kernel — all heavy matmuls in fp8e4 DoubleRow perf mode
(contraction-256 ops run in a single PE pass):
  - x ships as fp8 [128, 2, N] (the 2 = channel k-tiles) + the core's query
    half in fp32 (exact residual). GroupNorm stats are split across engines:
    channel tile 0 via DVE bn_stats, tile 1 via scalar-engine Copy/Square
    activations with accum_out. GN is folded into the QKV weights, which
    are scaled by 64 into fp8's healthy range.
  - q/k/v projections are single DoubleRow matmuls (contraction C=256 via
    k-tiles). v^T pairs 2..15 are produced inside chunk 0 of the attention
    loop (the projection PSUM bank is idle there), so the softmax pipeline
    starts ~10us earlier.
  - S^T is a plain fp8 K=32 matmul (the PE is power-throttled when array
    utilization is high, so avoiding redundant MACs matters more than
    packing). exp on the scalar engine, output fp8.
  - The softmax denominator is an M=1 ones DoubleRow matmul accumulated
    over key pairs (one PE column active - minimal array energy), then
    reciprocal_approx_fast + gpsimd partition_broadcast.
  - P*V accumulates key-block pairs per DoubleRow matmul into att2; the
    epilogue divides by the denominator during the fp8 evacuation
    (att8 = att2 * rec = 64*att), projects with fp8 DoubleRow, and fuses
    residual-add + 1/4096 rescale in one scalar_tensor_tensor.
"""
import sys

sys.path.insert(0, "/opt/trn_rl_repo")

import ml_dtypes
import numpy as np

import concourse.bass as bass
import concourse.tile as tile
from concourse import bacc, mybir
from concourse.bass_utils import run_bass_kernel_spmd

F32 = mybir.dt.float32
BF16 = mybir.dt.bfloat16
FP8 = mybir.dt.float8e4
DR = mybir.MatmulPerfMode.DoubleRow
MULT = mybir.AluOpType.mult
ADD = mybir.AluOpType.add

B, C, H, W = 4, 256, 64, 64
N = H * W          # 4096 tokens
NQ = N // 2        # 2048 query rows per core
D = C // 8         # 32 qk dim
G = 32             # groups
GS = C // G        # 8 channels per group
EPS = 1e-5
P = 128            # partitions
CT = C // P        # 2 channel tiles
CH = 512           # nq chunk
NCH = NQ // CH     # 4 chunks
MB = 128           # m block
NMB = N // MB      # 32 m blocks
NPR = NMB // 2     # 16 m-block pairs
WS = 64.0          # fp8 weight scale
SM_SCALE = float(D) ** -0.5
EXP_SCALE = SM_SCALE / (WS * WS)
NWARM = 32         # PE p-state warmup matmuls

_CACHE = {}
_last_in_maps = None


def _build():
    if "nc" in _CACHE:
        return _CACHE["nc"]

    nc = bacc.Bacc("TRN2", target_bir_lowering=False, debug=False, num_devices=8)

    WALL = 2 * D + 2 * C  # wq|wk|wv|wp columns, transposed, concatenated
    x8_ext = nc.declare_dram_parameter("x8", [P, CT * N], FP8, isOutput=False)
    xq_ext = nc.declare_dram_parameter("xq", [C, NQ], F32, isOutput=False)
    wall_ext = nc.declare_dram_parameter("wall", [C, WALL], F32, isOutput=False)
    bc4_ext = nc.declare_dram_parameter("bc4", [C, 4], F32, isOutput=False)  # gamma|beta|bv|bp
    bqk_ext = nc.declare_dram_parameter("bqk", [D, 2], F32, isOutput=False)  # bq|bk
    ind16_ext = nc.declare_dram_parameter("ind16", [P, G // CT], F32, isOutput=False)
    indb_ext = nc.declare_dram_parameter("indb", [G // CT, P], F32, isOutput=False)
    out_ext = nc.declare_dram_parameter("out", [C, NQ], F32, isOutput=True)

    GT = G // CT  # 16 groups per channel tile
    NHALF = N // 2

    with tile.TileContext(nc) as tc:
        with tc.tile_pool(name="const", bufs=1) as const, \
             tc.tile_pool(name="small", bufs=1) as small:
            ones8 = small.tile([P, 1], FP8, tag="ones8")
            nc.vector.memset(ones8, 1.0)

            # ---- input DMAs: x8 quadrants first, then weights, then xq ----
            x8 = const.tile([P, CT, N], FP8, tag="x8", name="x8")
            nc.sync.dma_start(out=x8[:, 0, 0:NHALF], in_=x8_ext[:, 0:NHALF])
            nc.scalar.dma_start(out=x8[:, 0, NHALF:N], in_=x8_ext[:, NHALF:N])
            nc.gpsimd.dma_start(out=x8[:, 1, 0:NHALF], in_=x8_ext[:, N:N + NHALF])
            nc.sync.dma_start(out=x8[:, 1, NHALF:N], in_=x8_ext[:, N + NHALF:2 * N])

            wall_sb = []
            for t in range(CT):
                cs = slice(t * P, (t + 1) * P)
                wl = const.tile([P, WALL], F32, tag=f"wall{t}", name=f"wall{t}")
                (nc.gpsimd if t == 0 else nc.scalar).dma_start(out=wl, in_=wall_ext[cs, :])
                wall_sb.append(wl)
            wqt_sb = [wall_sb[t][:, 0:D] for t in range(CT)]
            wkt_sb = [wall_sb[t][:, D:2 * D] for t in range(CT)]
            wvt_sb = [wall_sb[t][:, 2 * D:2 * D + C] for t in range(CT)]
            wpt_sb = [wall_sb[t][:, 2 * D + C:WALL] for t in range(CT)]

            bc4_sb = []
            for t in range(CT):
                cs = slice(t * P, (t + 1) * P)
                b4 = small.tile([P, 4], F32, tag=f"bc4{t}", name=f"bc4{t}")
                nc.sync.dma_start(out=b4, in_=bc4_ext[cs, :])
                bc4_sb.append(b4)
            gamma_sb = [bc4_sb[t][:, 0:1] for t in range(CT)]
            beta_sb = [bc4_sb[t][:, 1:2] for t in range(CT)]
            bv_sb = [bc4_sb[t][:, 2:3] for t in range(CT)]
            bp_sb = [bc4_sb[t][:, 3:4] for t in range(CT)]
            bqk_sb = small.tile([D, 2], F32, tag="bqk")
            nc.sync.dma_start(out=bqk_sb, in_=bqk_ext[:])
            bq_sb, bk_sb = bqk_sb[:, 0:1], bqk_sb[:, 1:2]
            ind16_sb = small.tile([P, GT], F32, tag="ind16")
            nc.sync.dma_start(out=ind16_sb, in_=ind16_ext[:])
            indb_sb = small.tile([GT, P], F32, tag="indb")
            nc.sync.dma_start(out=indb_sb, in_=indb_ext[:])
            eps_sb = small.tile([GT, 1], F32, tag="eps")
            nc.vector.memset(eps_sb, EPS)

            xq_f = []
            for t in range(CT):
                cs = slice(t * P, (t + 1) * P)
                xqt = const.tile([P, NQ], F32, tag=f"xqt{t}", name=f"xqt{t}")
                (nc.gpsimd if t == 0 else nc.scalar).dma_start(out=xqt, in_=xq_ext[cs, :])
                xq_f.append(xqt)

            xqb = [const.tile([P, NQ], F32, tag=f"xqb{t}", name=f"xqb{t}") for t in range(CT)]
            scale_sb = [small.tile([P, 1], F32, tag=f"scale{t}", name=f"scale{t}") for t in range(CT)]
            shift_sb = [small.tile([P, 1], F32, tag=f"shift{t}", name=f"shift{t}") for t in range(CT)]

            # ---- GroupNorm stats: tile 0 on DVE (bn_stats), tile 1 on the
            # scalar engine (Copy/Square + accum_out), in DMA-half granules ----
            with tc.tile_pool(name="gn", bufs=2) as gn, \
                 tc.tile_pool(name="gnps", bufs=1, space="PSUM") as gnps:
                mx_t = []
                # tile 0: DVE
                stats = gn.tile([P, 8, nc.vector.BN_STATS_DIM], F32, tag="st")
                for s in range(N // 512):
                    nc.vector.bn_stats(out=stats[:, s, :], in_=x8[:, 0, s * 512:(s + 1) * 512])
                mv = gn.tile([P, nc.vector.BN_AGGR_DIM], F32, tag="mv")
                nc.vector.bn_aggr(out=mv, in_=stats)
                mx0 = gn.tile([P, 2], F32, tag="mx0")
                nc.vector.tensor_copy(out=mx0[:, 0:1], in_=mv[:, 0:1])
                msq = gn.tile([P, 1], F32, tag="msq")
                nc.vector.tensor_mul(out=msq, in0=mv[:, 0:1], in1=mv[:, 0:1])
                nc.vector.tensor_add(out=mx0[:, 1:2], in0=mv[:, 1:2], in1=msq)
                mx_t.append(mx0)
                # tile 1: scalar engine accumulators
                scr = gn.tile([P, NHALF], FP8, tag="scr")
                acc = gn.tile([P, 4], F32, tag="acc")  # sum_h0|sum_h1|sq_h0|sq_h1
                for hh in range(2):
                    xs = x8[:, 1, hh * NHALF:(hh + 1) * NHALF]
                    nc.scalar.activation(
                        out=scr, in_=xs, func=mybir.ActivationFunctionType.Copy,
                        accum_out=acc[:, hh:hh + 1])
                for hh in range(2):
                    xs = x8[:, 1, hh * NHALF:(hh + 1) * NHALF]
                    nc.scalar.activation(
                        out=scr, in_=xs, func=mybir.ActivationFunctionType.Square,
                        accum_out=acc[:, 2 + hh:3 + hh])
                mx1 = gn.tile([P, 2], F32, tag="mx1")
                ssum = gn.tile([P, 2], F32, tag="ssum")
                nc.vector.tensor_add(out=ssum[:, 0:1], in0=acc[:, 0:1], in1=acc[:, 1:2])
                nc.vector.tensor_add(out=ssum[:, 1:2], in0=acc[:, 2:3], in1=acc[:, 3:4])
                nc.vector.tensor_scalar_mul(out=mx1, in0=ssum, scalar1=1.0 / float(N))
                mx_t.append(mx1)

                for t in range(CT):
                    gps = gnps.tile([GT, 2], F32, tag="gps")
                    nc.tensor.matmul(gps, ind16_sb, mx_t[t], start=True, stop=True)
                    gsb = gn.tile([GT, 2], F32, tag="gsb")
                    nc.vector.tensor_copy(out=gsb, in_=gps)
                    mg2 = gn.tile([GT, 1], F32, tag="mg2")
                    nc.vector.tensor_mul(out=mg2, in0=gsb[:, 0:1], in1=gsb[:, 0:1])
                    varg = gn.tile([GT, 1], F32, tag="varg")
                    nc.vector.tensor_sub(out=varg, in0=gsb[:, 1:2], in1=mg2)
                    sd = gn.tile([GT, 1], F32, tag="sd")
                    nc.scalar.activation(
                        out=sd, in_=varg,
                        func=mybir.ActivationFunctionType.Sqrt,
                        bias=eps_sb, scale=1.0,
                    )
                    g2 = gn.tile([GT, 2], F32, tag="g2")
                    nc.vector.tensor_copy(out=g2[:, 0:1], in_=gsb[:, 0:1])
                    nc.vector.reciprocal(out=g2[:, 1:2], in_=sd)

                    bc = gnps.tile([P, 2], F32, tag="bc")
                    nc.tensor.matmul(bc, indb_sb, g2, start=True, stop=True)
                    nc.vector.tensor_mul(out=scale_sb[t], in0=gamma_sb[t], in1=bc[:, 1:2])
                    sh1 = gn.tile([P, 1], F32, tag="sh1")
                    nc.vector.tensor_mul(out=sh1, in0=bc[:, 0:1], in1=scale_sb[t])
                    nc.vector.tensor_sub(out=shift_sb[t], in0=beta_sb[t], in1=sh1)

                # ---- fp8 weights (scaled by 64 * GN scale) + effective biases ----
                wq8 = const.tile([P, CT, D], FP8, tag="wq8")
                wk8 = const.tile([P, CT, D], FP8, tag="wk8")
                wv8 = const.tile([P, CT, C], FP8, tag="wv8")
                wp8 = const.tile([P, CT, C], FP8, tag="wp8")
                s64 = [small.tile([P, 1], F32, tag=f"s64_{t}", name=f"s64_{t}") for t in range(CT)]
                for t in range(CT):
                    nc.vector.tensor_scalar_mul(out=s64[t], in0=scale_sb[t], scalar1=WS)
                    nc.vector.tensor_scalar_mul(out=wq8[:, t, :], in0=wqt_sb[t], scalar1=s64[t])
                    nc.vector.tensor_scalar_mul(out=wk8[:, t, :], in0=wkt_sb[t], scalar1=s64[t])
                    nc.vector.tensor_scalar_mul(out=wv8[:, t, :], in0=wvt_sb[t], scalar1=s64[t])
                    nc.vector.tensor_scalar_mul(out=wp8[:, t, :], in0=wpt_sb[t], scalar1=WS)

                with tc.tile_pool(name="bps", bufs=1, space="PSUM") as bps:
                    bq64 = small.tile([D, 1], F32, tag="bq64")
                    bk64 = small.tile([D, 1], F32, tag="bk64")
                    psq = bps.tile([D, 1], F32, tag="pq")
                    psk = bps.tile([D, 1], F32, tag="pk")
                    for t in range(CT):
                        nc.tensor.matmul(psq, wqt_sb[t], shift_sb[t], start=(t == 0), stop=(t == CT - 1))
                        nc.tensor.matmul(psk, wkt_sb[t], shift_sb[t], start=(t == 0), stop=(t == CT - 1))
                    nc.vector.scalar_tensor_tensor(
                        out=bq64, in0=psq, scalar=1.0, in1=bq_sb, op0=MULT, op1=ADD)
                    nc.vector.tensor_scalar_mul(out=bq64, in0=bq64, scalar1=WS)
                    nc.vector.scalar_tensor_tensor(
                        out=bk64, in0=psk, scalar=1.0, in1=bk_sb, op0=MULT, op1=ADD)
                    nc.vector.tensor_scalar_mul(out=bk64, in0=bk64, scalar1=WS)

                    bv_eff = [small.tile([P, 1], F32, tag=f"bve{e}", name=f"bve{e}") for e in range(CT)]
                    for e in range(CT):
                        ps3 = bps.tile([P, 1], F32, tag=f"pv{e}", name=f"psv{e}")
                        for t in range(CT):
                            nc.tensor.matmul(
                                ps3, wvt_sb[t][:, e * P:(e + 1) * P], shift_sb[t],
                                start=(t == 0), stop=(t == CT - 1),
                            )
                        nc.vector.tensor_add(out=bv_eff[e], in0=ps3, in1=bv_sb[e])
                    for f in range(CT):
                        ps4 = bps.tile([P, 1], F32, tag=f"pp{f}", name=f"psp{f}")
                        for e in range(CT):
                            nc.tensor.matmul(
                                ps4, wpt_sb[e][:, f * P:(f + 1) * P], bv_eff[e],
                                start=(e == 0), stop=(e == CT - 1),
                            )
                        bp_eff = small.tile([P, 1], F32, tag=f"bpe{f}", name=f"bpe{f}")
                        nc.vector.tensor_add(out=bp_eff, in0=ps4, in1=bp_sb[f])
                        nc.vector.tensor_scalar_add(out=xqb[f], in0=xq_f[f], scalar1=bp_eff)

            # ---- q/k projections + v^T pairs 0/1 (rest made inside chunk 0) ----
            q8 = const.tile([D, NQ], FP8, tag="q8")
            k8 = const.tile([D, N], FP8, tag="k8")
            vt8 = const.tile([P, NMB, C], FP8, tag="vt8")

            def make_v_pair(pool, pr):
                vp = pool.tile([P, 2, C], F32, tag="pj", name=f"vp{pr}")
                for mloc in range(2):
                    mb = pr * 2 + mloc
                    ms = slice(mb * MB, (mb + 1) * MB)
                    nc.tensor.matmul(vp[:, mloc, :], x8[:, :, ms], wv8,
                                     start=True, stop=True, perf_mode=DR)
                nc.vector.tensor_copy(out=vt8[:, pr * 2:(pr + 1) * 2, :], in_=vp)

            with tc.tile_pool(name="qkps", bufs=2, space="PSUM") as qkps, \
                 tc.tile_pool(name="vtps", bufs=2, space="PSUM") as vtps:
                def project(dst, bias, w8, ns):
                    pr_ = qkps.tile([D, CH], F32, tag="kp", name="prj")
                    nc.tensor.matmul(pr_, w8, x8[:, :, ns], start=True, stop=True,
                                     perf_mode=DR)
                    nc.scalar.activation(
                        out=dst, in_=pr_,
                        func=mybir.ActivationFunctionType.Identity,
                        bias=bias, scale=1.0,
                    )

                project(q8[:, 0:CH], bq64, wq8, slice(0, CH))
                for ck in range(N // CH):
                    ns = slice(ck * CH, (ck + 1) * CH)
                    project(k8[:, ns], bk64, wk8, ns)
                for cq in range(1, NQ // CH):
                    ns = slice(cq * CH, (cq + 1) * CH)
                    project(q8[:, ns], bq64, wq8, ns)
                make_v_pair(vtps, 0)
                make_v_pair(vtps, 1)

            # ---- attention ----
            ones_b = ones8[:, None, :].broadcast_to([P, CT, 1])
            with tc.tile_pool(name="stps", bufs=2, space="PSUM") as stps, \
                 tc.tile_pool(name="attps", bufs=1, space="PSUM") as attps, \
                 tc.tile_pool(name="rsps", bufs=1, space="PSUM") as rsps, \
                 tc.tile_pool(name="pjps", bufs=1, space="PSUM") as pjps, \
                 tc.tile_pool(name="pp", bufs=6) as pp, \
                 tc.tile_pool(name="attsb", bufs=2) as attsb, \
                 tc.tile_pool(name="osb", bufs=4) as osb, \
                 tc.tile_pool(name="rsb", bufs=2) as rsb:
                pend = None

                def emit_epilogue(ep):
                    ns_p, att2_p, rs_p = ep
                    # att8 = att2 * 2^-6 = sum(P*v); the 1/denominator moves
                    # after the projection so the gpsimd broadcast overlaps
                    # the att evacuation + proj matmuls instead of gating them
                    att8 = attsb.tile([P, CT, CH], FP8, tag="att8")
                    for i in range(CT):
                        nc.vector.tensor_scalar_mul(
                            out=att8[:, i, :], in0=att2_p[:, i, :], scalar1=1.0 / WS)
                    rec1 = rsb.tile([1, CH], F32, tag="rec1")
                    nc.vector.reciprocal_approx_fast(out=rec1, in_=rs_p)
                    rec = rsb.tile([P, CH], F32, tag="rec")
                    nc.gpsimd.partition_broadcast(rec, rec1, channels=P)
                    for f in range(CT):
                        fs = slice(f * P, (f + 1) * P)
                        pj = pjps.tile([P, CH], F32, tag="pj", name=f"pj{f}")
                        nc.tensor.matmul(pj, wp8[:, :, fs], att8,
                                         start=True, stop=True, perf_mode=DR)
                        t1 = osb.tile([P, CH], F32, tag="t1")
                        nc.vector.tensor_mul(out=t1, in0=pj, in1=rec)
                        o = osb.tile([P, CH], F32, tag="o")
                        nc.vector.scalar_tensor_tensor(
                            out=o, in0=t1, scalar=1.0 / WS,
                            in1=xqb[f][:, ns_p], op0=MULT, op1=ADD)
                        nc.sync.dma_start(out=out_ext[fs, ns_p], in_=o)

                for ch in range(NCH):
                    ns = slice(ch * CH, (ch + 1) * CH)
                    att2 = attps.tile([P, CT, CH], F32, tag="att2")
                    rs = rsps.tile([1, CH], F32, tag="rs")
                    p_tiles = [None] * NPR
                    for g in range(NPR + 1):
                        if g < NPR:
                            stg = stps.tile([P, CT, CH], F32, tag="stg")
                            for j in range(2):
                                mb = g * 2 + j
                                nc.tensor.matmul(stg[:, j, :],
                                                 k8[:, mb * MB:(mb + 1) * MB],
                                                 q8[:, ns],
                                                 start=True, stop=True)
                            pg = pp.tile([P, CT, CH], FP8, tag="pg")
                            nc.scalar.activation(
                                out=pg, in_=stg,
                                func=mybir.ActivationFunctionType.Exp,
                                scale=EXP_SCALE,
                            )
                            p_tiles[g] = pg
                        if g == 1 and pend is not None:
                            emit_epilogue(pend)
                            pend = None
                        if g >= 1:
                            gp = g - 1
                            pg = p_tiles[gp]
                            nc.tensor.matmul(rs, ones_b, pg,
                                             start=(gp == 0), stop=(gp == NPR - 1),
                                             perf_mode=DR)
                            for e in range(CT):
                                nc.tensor.matmul(
                                    att2[:, e, :],
                                    vt8[:, 2 * gp:2 * gp + 2, e * P:(e + 1) * P],
                                    pg,
                                    start=(gp == 0), stop=(gp == NPR - 1),
                                    perf_mode=DR,
                                )
                            # chunk 0 doubles as the v^T production phase:
                            # pair gp+2 is built in the (otherwise idle) pj bank
                            if ch == 0 and gp + 2 < NPR:
                                make_v_pair(pjps, gp + 2)
                    pend = (ns, att2, rs)
                emit_epilogue(pend)

    nc.compile()
    _CACHE["nc"] = nc
    return nc


def _make_in_maps(x, gamma, beta, wq, bq, wk, bk, wv, bv, wp, bp):
    x = np.ascontiguousarray(np.asarray(x, dtype=np.float32))

    GT = G // CT
    ind16 = np.zeros((P, GT), np.float32)
    for c in range(P):
        ind16[c, c // GS] = 1.0 / GS
    indb = np.zeros((GT, P), np.float32)
    for c in range(P):
        indb[c // GS, c] = 1.0

    wall = np.concatenate(
        [
            np.asarray(wq, np.float32).T,
            np.asarray(wk, np.float32).T,
            np.asarray(wv, np.float32).T,
            np.asarray(wp, np.float32).T,
        ],
        axis=1,
    )
    bc4 = np.stack(
        [
            np.asarray(gamma, np.float32),
            np.asarray(beta, np.float32),
            np.asarray(bv, np.float32),
            np.asarray(bp, np.float32),
        ],
        axis=1,
    )
    bqk = np.stack([np.asarray(bq, np.float32), np.asarray(bk, np.float32)], axis=1)

    common = {
        "wall": np.ascontiguousarray(wall),
        "bc4": np.ascontiguousarray(bc4),
        "bqk": np.ascontiguousarray(bqk),
        "ind16": ind16,
        "indb": indb,
    }

    xf = x.reshape(B, C, N)
    # x8[p, t*N + n] = x[t*128+p, n] in fp8
    x8_all = np.ascontiguousarray(
        xf.reshape(B, CT, P, N).transpose(0, 2, 1, 3).reshape(B, P, CT * N)
    ).astype(ml_dtypes.float8_e4m3)
    in_maps = []
    for core in range(8):
        b, half = core // 2, core % 2
        m = dict(common)
        m["x8"] = x8_all[b]
        m["xq"] = np.ascontiguousarray(xf[b][:, half * NQ:(half + 1) * NQ])
        in_maps.append(m)
    return in_maps


def kernel(x, gamma, beta, wq, bq, wk, bk, wv, bv, wp, bp):
    nc = _build()
    in_maps = _make_in_maps(x, gamma, beta, wq, bq, wk, bk, wv, bv, wp, bp)
    global _last_in_maps
    _last_in_maps = in_maps
    res = run_bass_kernel_spmd(nc, in_maps, list(range(8)))

    y = np.empty((B, C, N), np.float32)
    for core in range(8):
        b, half = core // 2, core % 2
        y[b][:, half * NQ:(half + 1) * NQ] = res.results[core]["out"]
    return y.reshape(B, C, H, W)


# revision 7
# speedup vs baseline: 1.2237x; 1.2237x over previous
"""AttentionBlock (GroupNorm + 1x1-conv QKV self-attention + residual) on 8 TRN2 cores.

Sharding: data-parallel over batch B=4 x sequence-parallel over the 4096
tokens (2 cores per batch element, each handling 2048 query rows; K/V and
GroupNorm are computed redundantly per core pair — they are cheap relative
to attention).

Per-core device kernel — all heavy matmuls in fp8e4 DoubleRow perf mode
(contraction-256 ops run in a single PE pass):
  - x ships as fp8 [128, 2, N] (the 2 = channel k-tiles) + the core's query
    half in fp32 (exact residual). GroupNorm stats are split across engines:
    channel tile 0 via DVE bn_stats, tile 1 via scalar-engine Copy/Square
    activations with accum_out. GN is folded into the QKV weights, which
    are scaled by 64 into fp8's healthy range.
  - q/k/v projections are single DoubleRow matmuls (contraction C=256 via
    k-tiles). v^T pairs 2..15 are produced inside chunk 0 of the attention
    loop (the projection PSUM bank is idle there), so the softmax pipeline
    starts ~10us earlier.
  - S^T is a plain fp8 K=32 matmul (the PE is power-throttled when array
    utilization is high, so avoiding redundant MACs matters more than
    packing). exp on the scalar engine, output fp8.
  - The softmax denominator is an M=1 ones DoubleRow matmul accumulated
    over key pairs (one PE column active - minimal array energy), then
    reciprocal_approx_fast + gpsimd partition_broadcast.
  - P*V accumulates key-block pairs per DoubleRow matmul into att2; the
    epilogue divides by the denominator during the fp8 evacuation
    (att8 = att2 * rec = 64*att), projects with fp8 DoubleRow, and fuses
    residual-add + 1/4096 rescale in one scalar_tensor_tensor.
"""
import sys

sys.path.insert(0, "/opt/trn_rl_repo")

import ml_dtypes
import numpy as np

import concourse.bass as bass
import concourse.tile as tile
from concourse import bacc, mybir
from concourse.bass_utils import run_bass_kernel_spmd

F32 = mybir.dt.float32
BF16 = mybir.dt.bfloat16
FP8 = mybir.dt.float8e4
DR = mybir.MatmulPerfMode.DoubleRow
MULT = mybir.AluOpType.mult
ADD = mybir.AluOpType.add

B, C, H, W = 4, 256, 64, 64
N = H * W          # 4096 tokens
NQ = N // 2        # 2048 query rows per core
D = C // 8         # 32 qk dim
G = 32             # groups
GS = C // G        # 8 channels per group
EPS = 1e-5
P = 128            # partitions
CT = C // P        # 2 channel tiles
CH = 512           # nq chunk
NCH = NQ // CH     # 4 chunks
MB = 128           # m block
NMB = N // MB      # 32 m blocks
NPR = NMB // 2     # 16 m-block pairs
WS = 64.0          # fp8 weight scale
SM_SCALE = float(D) ** -0.5
EXP_SCALE = SM_SCALE / (WS * WS)
NWARM = 32         # PE p-state warmup matmuls

_CACHE = {}
_last_in_maps = None


def _build():
    if "nc" in _CACHE:
        return _CACHE["nc"]

    nc = bacc.Bacc("TRN2", target_bir_lowering=False, debug=False, num_devices=8)

    WALL = 2 * D + 2 * C  # wq|wk|wv|wp columns, transposed, concatenated
    x8_ext = nc.declare_dram_parameter("x8", [P, CT * N], FP8, isOutput=False)
    xq_ext = nc.declare_dram_parameter("xq", [C, NQ], F32, isOutput=False)
    wall_ext = nc.declare_dram_parameter("wall", [C, WALL], F32, isOutput=False)
    bc4_ext = nc.declare_dram_parameter("bc4", [C, 4], F32, isOutput=False)  # gamma|beta|bv|bp
    bqk_ext = nc.declare_dram_parameter("bqk", [D, 2], F32, isOutput=False)  # bq|bk
    ind16_ext = nc.declare_dram_parameter("ind16", [P, G // CT], F32, isOutput=False)
    indb_ext = nc.declare_dram_parameter("indb", [G // CT, P], F32, isOutput=False)
    out_ext = nc.declare_dram_parameter("out", [C, NQ], F32, isOutput=True)

    GT = G // CT  # 16 groups per channel tile
    NHALF = N // 2

    with tile.TileContext(nc) as tc:
        with tc.tile_pool(name="const", bufs=1) as const, \
             tc.tile_pool(name="small", bufs=1) as small:
            ones8 = small.tile([P, 1], FP8, tag="ones8")
            nc.vector.memset(ones8, 1.0)

            # ---- input DMAs: x8 quadrants first, then weights, then xq ----
            x8 = const.tile([P, CT, N], FP8, tag="x8", name="x8")
            nc.sync.dma_start(out=x8[:, 0, 0:NHALF], in_=x8_ext[:, 0:NHALF])
            nc.scalar.dma_start(out=x8[:, 0, NHALF:N], in_=x8_ext[:, NHALF:N])
            nc.gpsimd.dma_start(out=x8[:, 1, 0:NHALF], in_=x8_ext[:, N:N + NHALF])
            nc.sync.dma_start(out=x8[:, 1, NHALF:N], in_=x8_ext[:, N + NHALF:2 * N])

            wall_sb = []
            for t in range(CT):
                cs = slice(t * P, (t + 1) * P)
                wl = const.tile([P, WALL], F32, tag=f"wall{t}", name=f"wall{t}")
                (nc.gpsimd if t == 0 else nc.scalar).dma_start(out=wl, in_=wall_ext[cs, :])
                wall_sb.append(wl)
            wqt_sb = [wall_sb[t][:, 0:D] for t in range(CT)]
            wkt_sb = [wall_sb[t][:, D:2 * D] for t in range(CT)]
            wvt_sb = [wall_sb[t][:, 2 * D:2 * D + C] for t in range(CT)]
            wpt_sb = [wall_sb[t][:, 2 * D + C:WALL] for t in range(CT)]

            bc4_sb = []
            for t in range(CT):
                cs = slice(t * P, (t + 1) * P)
                b4 = small.tile([P, 4], F32, tag=f"bc4{t}", name=f"bc4{t}")
                nc.sync.dma_start(out=b4, in_=bc4_ext[cs, :])
                bc4_sb.append(b4)
            gamma_sb = [bc4_sb[t][:, 0:1] for t in range(CT)]
            beta_sb = [bc4_sb[t][:, 1:2] for t in range(CT)]
            bv_sb = [bc4_sb[t][:, 2:3] for t in range(CT)]
            bp_sb = [bc4_sb[t][:, 3:4] for t in range(CT)]
            bqk_sb = small.tile([D, 2], F32, tag="bqk")
            nc.sync.dma_start(out=bqk_sb, in_=bqk_ext[:])
            bq_sb, bk_sb = bqk_sb[:, 0:1], bqk_sb[:, 1:2]
            ind16_sb = small.tile([P, GT], F32, tag="ind16")
            nc.sync.dma_start(out=ind16_sb, in_=ind16_ext[:])
            indb_sb = small.tile([GT, P], F32, tag="indb")
            nc.sync.dma_start(out=indb_sb, in_=indb_ext[:])
            eps_sb = small.tile([GT, 1], F32, tag="eps")
            nc.vector.memset(eps_sb, EPS)

            xq_f = []
            for t in range(CT):
                cs = slice(t * P, (t + 1) * P)
                xqt = const.tile([P, NQ], F32, tag=f"xqt{t}", name=f"xqt{t}")
                (nc.gpsimd if t == 0 else nc.scalar).dma_start(out=xqt, in_=xq_ext[cs, :])
                xq_f.append(xqt)

            xqb = [const.tile([P, NQ], F32, tag=f"xqb{t}", name=f"xqb{t}") for t in range(CT)]
            scale_sb = [small.tile([P, 1], F32, tag=f"scale{t}", name=f"scale{t}") for t in range(CT)]
            shift_sb = [small.tile([P, 1], F32, tag=f"shift{t}", name=f"shift{t}") for t in range(CT)]

            # ---- GroupNorm stats: tile 0 on DVE (bn_stats), tile 1 on the
            # scalar engine (Copy/Square + accum_out), in DMA-half granules ----
            with tc.tile_pool(name="gn", bufs=2) as gn, \
                 tc.tile_pool(name="gnps", bufs=1, space="PSUM") as gnps:
                mx_t = []
                # tile 0 (8 pieces) + first 2 pieces of tile 1: DVE bn_stats;
                # remaining 6 pieces of tile 1: scalar Copy/Square + accum_out.
                # Balances ~6.8us on each engine while the x8 DMA streams in.
                stats = gn.tile([P, 8, nc.vector.BN_STATS_DIM], F32, tag="st")
                for s in range(N // 512):
                    nc.vector.bn_stats(out=stats[:, s, :], in_=x8[:, 0, s * 512:(s + 1) * 512])
                statsb = gn.tile([P, 2, nc.vector.BN_STATS_DIM], F32, tag="stb")
                for s in range(2):
                    nc.vector.bn_stats(out=statsb[:, s, :], in_=x8[:, 1, s * 512:(s + 1) * 512])
                scr = gn.tile([P, NHALF], FP8, tag="scr")
                acc = gn.tile([P, 4], F32, tag="acc")  # sum_a|sum_b|sq_a|sq_b
                nc.scalar.activation(
                    out=scr[:, 0:1024], in_=x8[:, 1, 1024:2048],
                    func=mybir.ActivationFunctionType.Copy, accum_out=acc[:, 0:1])
                nc.scalar.activation(
                    out=scr, in_=x8[:, 1, 2048:N],
                    func=mybir.ActivationFunctionType.Copy, accum_out=acc[:, 1:2])
                nc.scalar.activation(
                    out=scr[:, 0:1024], in_=x8[:, 1, 1024:2048],
                    func=mybir.ActivationFunctionType.Square, accum_out=acc[:, 2:3])
                nc.scalar.activation(
                    out=scr, in_=x8[:, 1, 2048:N],
                    func=mybir.ActivationFunctionType.Square, accum_out=acc[:, 3:4])
                mv = gn.tile([P, nc.vector.BN_AGGR_DIM], F32, tag="mv")
                nc.vector.bn_aggr(out=mv, in_=stats)
                mx0 = gn.tile([P, 2], F32, tag="mx0")
                nc.vector.tensor_copy(out=mx0[:, 0:1], in_=mv[:, 0:1])
                msq = gn.tile([P, 1], F32, tag="msq")
                nc.vector.tensor_mul(out=msq, in0=mv[:, 0:1], in1=mv[:, 0:1])
                nc.vector.tensor_add(out=mx0[:, 1:2], in0=mv[:, 1:2], in1=msq)
                mx_t.append(mx0)
                # combine tile-1 DVE piece (1024 cols) with scalar pieces (3072)
                mvb = gn.tile([P, nc.vector.BN_AGGR_DIM], F32, tag="mvb")
                nc.vector.bn_aggr(out=mvb, in_=statsb)
                sumb = gn.tile([P, 2], F32, tag="sumb")  # sum | sumsq of DVE share
                nc.vector.tensor_scalar_mul(out=sumb[:, 0:1], in0=mvb[:, 0:1], scalar1=1024.0)
                msqb = gn.tile([P, 1], F32, tag="msqb")
                nc.vector.tensor_mul(out=msqb, in0=mvb[:, 0:1], in1=mvb[:, 0:1])
                nc.vector.tensor_add(out=msqb, in0=msqb, in1=mvb[:, 1:2])
                nc.vector.tensor_scalar_mul(out=sumb[:, 1:2], in0=msqb, scalar1=1024.0)
                mx1 = gn.tile([P, 2], F32, tag="mx1")
                ssum = gn.tile([P, 2], F32, tag="ssum")
                nc.vector.tensor_add(out=ssum[:, 0:1], in0=acc[:, 0:1], in1=acc[:, 1:2])
                nc.vector.tensor_add(out=ssum[:, 1:2], in0=acc[:, 2:3], in1=acc[:, 3:4])
                nc.vector.tensor_add(out=ssum, in0=ssum, in1=sumb)
                nc.vector.tensor_scalar_mul(out=mx1, in0=ssum, scalar1=1.0 / float(N))
                mx_t.append(mx1)

                for t in range(CT):
                    gps = gnps.tile([GT, 2], F32, tag="gps")
                    nc.tensor.matmul(gps, ind16_sb, mx_t[t], start=True, stop=True)
                    gsb = gn.tile([GT, 2], F32, tag="gsb")
                    nc.vector.tensor_copy(out=gsb, in_=gps)
                    mg2 = gn.tile([GT, 1], F32, tag="mg2")
                    nc.vector.tensor_mul(out=mg2, in0=gsb[:, 0:1], in1=gsb[:, 0:1])
                    varg = gn.tile([GT, 1], F32, tag="varg")
                    nc.vector.tensor_sub(out=varg, in0=gsb[:, 1:2], in1=mg2)
                    sd = gn.tile([GT, 1], F32, tag="sd")
                    nc.scalar.activation(
                        out=sd, in_=varg,
                        func=mybir.ActivationFunctionType.Sqrt,
                        bias=eps_sb, scale=1.0,
                    )
                    g2 = gn.tile([GT, 2], F32, tag="g2")
                    nc.vector.tensor_copy(out=g2[:, 0:1], in_=gsb[:, 0:1])
                    nc.vector.reciprocal(out=g2[:, 1:2], in_=sd)

                    bc = gnps.tile([P, 2], F32, tag="bc")
                    nc.tensor.matmul(bc, indb_sb, g2, start=True, stop=True)
                    nc.vector.tensor_mul(out=scale_sb[t], in0=gamma_sb[t], in1=bc[:, 1:2])
                    sh1 = gn.tile([P, 1], F32, tag="sh1")
                    nc.vector.tensor_mul(out=sh1, in0=bc[:, 0:1], in1=scale_sb[t])
                    nc.vector.tensor_sub(out=shift_sb[t], in0=beta_sb[t], in1=sh1)

                # ---- fp8 weights (scaled by 64 * GN scale) + effective biases ----
                wq8 = const.tile([P, CT, D], FP8, tag="wq8")
                wk8 = const.tile([P, CT, D], FP8, tag="wk8")
                wv8 = const.tile([P, CT, C], FP8, tag="wv8")
                wp8 = const.tile([P, CT, C], FP8, tag="wp8")
                s64 = [small.tile([P, 1], F32, tag=f"s64_{t}", name=f"s64_{t}") for t in range(CT)]
                for t in range(CT):
                    nc.vector.tensor_scalar_mul(out=s64[t], in0=scale_sb[t], scalar1=WS)
                    nc.vector.tensor_scalar_mul(out=wq8[:, t, :], in0=wqt_sb[t], scalar1=s64[t])
                    nc.vector.tensor_scalar_mul(out=wk8[:, t, :], in0=wkt_sb[t], scalar1=s64[t])
                    nc.vector.tensor_scalar_mul(out=wv8[:, t, :], in0=wvt_sb[t], scalar1=s64[t])
                    nc.vector.tensor_scalar_mul(out=wp8[:, t, :], in0=wpt_sb[t], scalar1=WS)

                with tc.tile_pool(name="bps", bufs=1, space="PSUM") as bps:
                    bq64 = small.tile([D, 1], F32, tag="bq64")
                    bk64 = small.tile([D, 1], F32, tag="bk64")
                    psq = bps.tile([D, 1], F32, tag="pq")
                    psk = bps.tile([D, 1], F32, tag="pk")
                    for t in range(CT):
                        nc.tensor.matmul(psq, wqt_sb[t], shift_sb[t], start=(t == 0), stop=(t == CT - 1))
                        nc.tensor.matmul(psk, wkt_sb[t], shift_sb[t], start=(t == 0), stop=(t == CT - 1))
                    nc.vector.scalar_tensor_tensor(
                        out=bq64, in0=psq, scalar=1.0, in1=bq_sb, op0=MULT, op1=ADD)
                    nc.vector.tensor_scalar_mul(out=bq64, in0=bq64, scalar1=WS)
                    nc.vector.scalar_tensor_tensor(
                        out=bk64, in0=psk, scalar=1.0, in1=bk_sb, op0=MULT, op1=ADD)
                    nc.vector.tensor_scalar_mul(out=bk64, in0=bk64, scalar1=WS)

                    bv_eff = [small.tile([P, 1], F32, tag=f"bve{e}", name=f"bve{e}") for e in range(CT)]
                    for e in range(CT):
                        ps3 = bps.tile([P, 1], F32, tag=f"pv{e}", name=f"psv{e}")
                        for t in range(CT):
                            nc.tensor.matmul(
                                ps3, wvt_sb[t][:, e * P:(e + 1) * P], shift_sb[t],
                                start=(t == 0), stop=(t == CT - 1),
                            )
                        nc.vector.tensor_add(out=bv_eff[e], in0=ps3, in1=bv_sb[e])
                    for f in range(CT):
                        ps4 = bps.tile([P, 1], F32, tag=f"pp{f}", name=f"psp{f}")
                        for e in range(CT):
                            nc.tensor.matmul(
                                ps4, wpt_sb[e][:, f * P:(f + 1) * P], bv_eff[e],
                                start=(e == 0), stop=(e == CT - 1),
                            )
                        bp_eff = small.tile([P, 1], F32, tag=f"bpe{f}", name=f"bpe{f}")
                        nc.vector.tensor_add(out=bp_eff, in0=ps4, in1=bp_sb[f])
                        nc.vector.tensor_scalar_add(out=xqb[f], in0=xq_f[f], scalar1=bp_eff)

            # ---- q/k projections + v^T pairs 0/1 (rest made inside chunk 0) ----
            q8 = const.tile([D, NQ], FP8, tag="q8")
            k8 = const.tile([D, N], FP8, tag="k8")
            vt8 = const.tile([P, NMB, C], FP8, tag="vt8")

            def make_v_pair(pool, pr):
                vp = pool.tile([P, 2, C], F32, tag="pj", name=f"vp{pr}")
                for mloc in range(2):
                    mb = pr * 2 + mloc
                    ms = slice(mb * MB, (mb + 1) * MB)
                    nc.tensor.matmul(vp[:, mloc, :], x8[:, :, ms], wv8,
                                     start=True, stop=True, perf_mode=DR)
                nc.vector.tensor_copy(out=vt8[:, pr * 2:(pr + 1) * 2, :], in_=vp)

            with tc.tile_pool(name="qkps", bufs=2, space="PSUM") as qkps, \
                 tc.tile_pool(name="vtps", bufs=2, space="PSUM") as vtps:
                def project(dst, bias, w8, ns):
                    pr_ = qkps.tile([D, CH], F32, tag="kp", name="prj")
                    nc.tensor.matmul(pr_, w8, x8[:, :, ns], start=True, stop=True,
                                     perf_mode=DR)
                    nc.scalar.activation(
                        out=dst, in_=pr_,
                        func=mybir.ActivationFunctionType.Identity,
                        bias=bias, scale=1.0,
                    )

                project(q8[:, 0:CH], bq64, wq8, slice(0, CH))
                for ck in range(N // CH):
                    ns = slice(ck * CH, (ck + 1) * CH)
                    project(k8[:, ns], bk64, wk8, ns)
                for cq in range(1, NQ // CH):
                    ns = slice(cq * CH, (cq + 1) * CH)
                    project(q8[:, ns], bq64, wq8, ns)
                make_v_pair(vtps, 0)
                make_v_pair(vtps, 1)

            # ---- attention ----
            ones_b = ones8[:, None, :].broadcast_to([P, CT, 1])
            with tc.tile_pool(name="stps", bufs=2, space="PSUM") as stps, \
                 tc.tile_pool(name="attps", bufs=1, space="PSUM") as attps, \
                 tc.tile_pool(name="rsps", bufs=1, space="PSUM") as rsps, \
                 tc.tile_pool(name="pjps", bufs=1, space="PSUM") as pjps, \
                 tc.tile_pool(name="pp", bufs=6) as pp, \
                 tc.tile_pool(name="attsb", bufs=2) as attsb, \
                 tc.tile_pool(name="osb", bufs=4) as osb, \
                 tc.tile_pool(name="rsb", bufs=2) as rsb:
                pend = None

                def epilogue_a(ep):
                    # DVE-only: att8 = att2 * 2^-6 = sum(P*v) (fp8 cast), and
                    # the denominator reciprocal + gpsimd broadcast. The
                    # 1/denominator is applied after the projection so this
                    # never gates the PE.
                    ns_p, att2_p, rs_p = ep
                    att8 = attsb.tile([P, CT, CH], FP8, tag="att8")
                    for i in range(CT):
                        nc.vector.tensor_scalar_mul(
                            out=att8[:, i, :], in0=att2_p[:, i, :], scalar1=1.0 / WS)
                    rec1 = rsb.tile([1, CH], F32, tag="rec1")
                    nc.vector.reciprocal_approx_fast(out=rec1, in_=rs_p)
                    rec = rsb.tile([P, CH], F32, tag="rec")
                    nc.gpsimd.partition_broadcast(rec, rec1, channels=P)
                    return att8, rec

                def epilogue_b(ep, att8, rec):
                    # PE projection + fused rescale/residual + output DMA,
                    # emitted two pairs later so att8 is ready when the
                    # in-order PE reaches the proj matmuls.
                    ns_p, att2_p, rs_p = ep
                    for f in range(CT):
                        fs = slice(f * P, (f + 1) * P)
                        pj = pjps.tile([P, CH], F32, tag="pj", name=f"pj{f}")
                        nc.tensor.matmul(pj, wp8[:, :, fs], att8,
                                         start=True, stop=True, perf_mode=DR)
                        t1 = osb.tile([P, CH], F32, tag="t1")
                        nc.vector.tensor_mul(out=t1, in0=pj, in1=rec)
                        o = osb.tile([P, CH], F32, tag="o")
                        nc.vector.scalar_tensor_tensor(
                            out=o, in0=t1, scalar=1.0 / WS,
                            in1=xqb[f][:, ns_p], op0=MULT, op1=ADD)
                        nc.sync.dma_start(out=out_ext[fs, ns_p], in_=o)

                for ch in range(NCH):
                    ns = slice(ch * CH, (ch + 1) * CH)
                    att2 = attps.tile([P, CT, CH], F32, tag="att2")
                    rs = rsps.tile([1, CH], F32, tag="rs")
                    p_tiles = [None] * NPR
                    for g in range(NPR + 1):
                        if g < NPR:
                            stg = stps.tile([P, CT, CH], F32, tag="stg")
                            for j in range(2):
                                mb = g * 2 + j
                                nc.tensor.matmul(stg[:, j, :],
                                                 k8[:, mb * MB:(mb + 1) * MB],
                                                 q8[:, ns],
                                                 start=True, stop=True)
                            pg = pp.tile([P, CT, CH], FP8, tag="pg")
                            nc.scalar.activation(
                                out=pg, in_=stg,
                                func=mybir.ActivationFunctionType.Exp,
                                scale=EXP_SCALE,
                            )
                            p_tiles[g] = pg
                        if g == 1 and pend is not None:
                            ep_state = epilogue_a(pend)
                        if g == 3 and pend is not None:
                            epilogue_b(pend, *ep_state)
                            pend = None
                        if g >= 1:
                            gp = g - 1
                            pg = p_tiles[gp]
                            nc.tensor.matmul(rs, ones_b, pg,
                                             start=(gp == 0), stop=(gp == NPR - 1),
                                             perf_mode=DR)
                            for e in range(CT):
                                nc.tensor.matmul(
                                    att2[:, e, :],
                                    vt8[:, 2 * gp:2 * gp + 2, e * P:(e + 1) * P],
                                    pg,
                                    start=(gp == 0), stop=(gp == NPR - 1),
                                    perf_mode=DR,
                                )
                            # chunk 0 doubles as the v^T production phase:
                            # pair gp+2 is built in the (otherwise idle) pj bank
                            if ch == 0 and gp + 2 < NPR:
                                make_v_pair(pjps, gp + 2)
                    pend = (ns, att2, rs)
                ep_state = epilogue_a(pend)
                epilogue_b(pend, *ep_state)

    nc.compile()
    _CACHE["nc"] = nc
    return nc


def _make_in_maps(x, gamma, beta, wq, bq, wk, bk, wv, bv, wp, bp):
    x = np.ascontiguousarray(np.asarray(x, dtype=np.float32))

    GT = G // CT
    ind16 = np.zeros((P, GT), np.float32)
    for c in range(P):
        ind16[c, c // GS] = 1.0 / GS
    indb = np.zeros((GT, P), np.float32)
    for c in range(P):
        indb[c // GS, c] = 1.0

    wall = np.concatenate(
        [
            np.asarray(wq, np.float32).T,
            np.asarray(wk, np.float32).T,
            np.asarray(wv, np.float32).T,
            np.asarray(wp, np.float32).T,
        ],
        axis=1,
    )
    bc4 = np.stack(
        [
            np.asarray(gamma, np.float32),
            np.asarray(beta, np.float32),
            np.asarray(bv, np.float32),
            np.asarray(bp, np.float32),
        ],
        axis=1,
    )
    bqk = np.stack([np.asarray(bq, np.float32), np.asarray(bk, np.float32)], axis=1)

    common = {
        "wall": np.ascontiguousarray(wall),
        "bc4": np.ascontiguousarray(bc4),
        "bqk": np.ascontiguousarray(bqk),
        "ind16": ind16,
        "indb": indb,
    }

    xf = x.reshape(B, C, N)
    # x8[p, t*N + n] = x[t*128+p, n] in fp8
    x8_all = np.ascontiguousarray(
        xf.reshape(B, CT, P, N).transpose(0, 2, 1, 3).reshape(B, P, CT * N)
    ).astype(ml_dtypes.float8_e4m3)
    in_maps = []
    for core in range(8):
        b, half = core // 2, core % 2
        m = dict(common)
        m["x8"] = x8_all[b]
        m["xq"] = np.ascontiguousarray(xf[b][:, half * NQ:(half + 1) * NQ])
        in_maps.append(m)
    return in_maps


def kernel(x, gamma, beta, wq, bq, wk, bk, wv, bv, wp, bp):
    nc = _build()
    in_maps = _make_in_maps(x, gamma, beta, wq, bq, wk, bk, wv, bv, wp, bp)
    global _last_in_maps
    _last_in_maps = in_maps
    res = run_bass_kernel_spmd(nc, in_maps, list(range(8)))

    y = np.empty((B, C, N), np.float32)
    for core in range(8):
        b, half = core // 2, core % 2
        y[b][:, half * NQ:(half + 1) * NQ] = res.results[core]["out"]
    return y.reshape(B, C, H, W)


# revision 8
# speedup vs baseline: 1.2761x; 1.0428x over previous
"""AttentionBlock (GroupNorm + 1x1-conv QKV self-attention + residual) on 8 TRN2 cores.

Sharding: data-parallel over batch B=4 x sequence-parallel over the 4096
tokens (2 cores per batch element, each handling 2048 query rows; K/V and
GroupNorm are computed redundantly per core pair — they are cheap relative
to attention).

Per-core device kernel — all heavy matmuls in fp8e4 DoubleRow perf mode
(contraction-256 ops run in a single PE pass):
  - x ships as fp8 [128, 2, N] (the 2 = channel k-tiles) + the core's query
    half in fp32 (exact residual). GroupNorm stats are split across engines:
    channel tile 0 via DVE bn_stats, tile 1 via scalar-engine Copy/Square
    activations with accum_out. GN is folded into the QKV weights, which
    are scaled by 64 into fp8's healthy range.
  - q/k/v projections are single DoubleRow matmuls (contraction C=256 via
    k-tiles). v^T pairs 2..15 are produced inside chunk 0 of the attention
    loop (the projection PSUM bank is idle there), so the softmax pipeline
    starts ~10us earlier.
  - S^T is a plain fp8 K=32 matmul (the PE is power-throttled when array
    utilization is high, so avoiding redundant MACs matters more than
    packing). exp on the scalar engine, output fp8.
  - The softmax denominator is an M=1 ones DoubleRow matmul accumulated
    over key pairs (one PE column active - minimal array energy), then
    reciprocal_approx_fast + gpsimd partition_broadcast.
  - P*V accumulates key-block pairs per DoubleRow matmul into att2; the
    epilogue divides by the denominator during the fp8 evacuation
    (att8 = att2 * rec = 64*att), projects with fp8 DoubleRow, and fuses
    residual-add + 1/4096 rescale in one scalar_tensor_tensor.
"""
import sys

sys.path.insert(0, "/opt/trn_rl_repo")

import ml_dtypes
import numpy as np

import concourse.bass as bass
import concourse.tile as tile
from concourse import bacc, mybir
from concourse.bass_utils import run_bass_kernel_spmd

F32 = mybir.dt.float32
BF16 = mybir.dt.bfloat16
FP8 = mybir.dt.float8e4
DR = mybir.MatmulPerfMode.DoubleRow
MULT = mybir.AluOpType.mult
ADD = mybir.AluOpType.add

B, C, H, W = 4, 256, 64, 64
N = H * W          # 4096 tokens
NQ = N // 2        # 2048 query rows per core
D = C // 8         # 32 qk dim
G = 32             # groups
GS = C // G        # 8 channels per group
EPS = 1e-5
P = 128            # partitions
CT = C // P        # 2 channel tiles
CH = 512           # nq chunk
NCH = NQ // CH     # 4 chunks
MB = 128           # m block
NMB = N // MB      # 32 m blocks
NPR = NMB // 2     # 16 m-block pairs
WS = 64.0          # fp8 weight scale
SM_SCALE = float(D) ** -0.5
EXP_SCALE = SM_SCALE / (WS * WS)
NWARM = 32         # PE p-state warmup matmuls

_CACHE = {}
_last_in_maps = None


def _build():
    if "nc" in _CACHE:
        return _CACHE["nc"]

    nc = bacc.Bacc("TRN2", target_bir_lowering=False, debug=False, num_devices=8)

    WALL = 2 * D + 2 * C  # wq|wk|wv|wp columns, transposed, concatenated
    x8_ext = nc.declare_dram_parameter("x8", [P, CT * N], FP8, isOutput=False)
    xq_ext = nc.declare_dram_parameter("xq", [C, NQ], F32, isOutput=False)
    wall_ext = nc.declare_dram_parameter("wall", [C, WALL], F32, isOutput=False)
    bc4_ext = nc.declare_dram_parameter("bc4", [C, 4], F32, isOutput=False)  # gamma|beta|bv|bp
    bqk_ext = nc.declare_dram_parameter("bqk", [D, 2], F32, isOutput=False)  # bq|bk
    ind16_ext = nc.declare_dram_parameter("ind16", [P, G // CT], F32, isOutput=False)
    indb_ext = nc.declare_dram_parameter("indb", [G // CT, P], F32, isOutput=False)
    out_ext = nc.declare_dram_parameter("out", [C, NQ], F32, isOutput=True)

    GT = G // CT  # 16 groups per channel tile
    NHALF = N // 2

    with tile.TileContext(nc) as tc:
        with tc.tile_pool(name="const", bufs=1) as const, \
             tc.tile_pool(name="small", bufs=1) as small:
            ones8 = small.tile([P, 1], FP8, tag="ones8")
            nc.vector.memset(ones8, 1.0)

            # ---- input DMAs: x8 quadrants first, then weights, then xq ----
            x8 = const.tile([P, CT, N], FP8, tag="x8", name="x8")
            nc.sync.dma_start(out=x8[:, 0, 0:NHALF], in_=x8_ext[:, 0:NHALF])
            nc.scalar.dma_start(out=x8[:, 0, NHALF:N], in_=x8_ext[:, NHALF:N])
            nc.gpsimd.dma_start(out=x8[:, 1, 0:NHALF], in_=x8_ext[:, N:N + NHALF])
            nc.sync.dma_start(out=x8[:, 1, NHALF:N], in_=x8_ext[:, N + NHALF:2 * N])

            wall_sb = []
            for t in range(CT):
                cs = slice(t * P, (t + 1) * P)
                wl = const.tile([P, WALL], F32, tag=f"wall{t}", name=f"wall{t}")
                (nc.gpsimd if t == 0 else nc.scalar).dma_start(out=wl, in_=wall_ext[cs, :])
                wall_sb.append(wl)
            wqt_sb = [wall_sb[t][:, 0:D] for t in range(CT)]
            wkt_sb = [wall_sb[t][:, D:2 * D] for t in range(CT)]
            wvt_sb = [wall_sb[t][:, 2 * D:2 * D + C] for t in range(CT)]
            wpt_sb = [wall_sb[t][:, 2 * D + C:WALL] for t in range(CT)]

            bc4_sb = []
            for t in range(CT):
                cs = slice(t * P, (t + 1) * P)
                b4 = small.tile([P, 4], F32, tag=f"bc4{t}", name=f"bc4{t}")
                nc.sync.dma_start(out=b4, in_=bc4_ext[cs, :])
                bc4_sb.append(b4)
            gamma_sb = [bc4_sb[t][:, 0:1] for t in range(CT)]
            beta_sb = [bc4_sb[t][:, 1:2] for t in range(CT)]
            bv_sb = [bc4_sb[t][:, 2:3] for t in range(CT)]
            bp_sb = [bc4_sb[t][:, 3:4] for t in range(CT)]
            bqk_sb = small.tile([D, 2], F32, tag="bqk")
            nc.sync.dma_start(out=bqk_sb, in_=bqk_ext[:])
            bq_sb, bk_sb = bqk_sb[:, 0:1], bqk_sb[:, 1:2]
            ind16_sb = small.tile([P, GT], F32, tag="ind16")
            nc.sync.dma_start(out=ind16_sb, in_=ind16_ext[:])
            indb_sb = small.tile([GT, P], F32, tag="indb")
            nc.sync.dma_start(out=indb_sb, in_=indb_ext[:])
            eps_sb = small.tile([GT, 1], F32, tag="eps")
            nc.vector.memset(eps_sb, EPS)

            xqb = [const.tile([P, NQ], F32, tag=f"xqb{t}", name=f"xqb{t}") for t in range(CT)]
            scale_sb = [small.tile([P, 1], F32, tag=f"scale{t}", name=f"scale{t}") for t in range(CT)]
            shift_sb = [small.tile([P, 1], F32, tag=f"shift{t}", name=f"shift{t}") for t in range(CT)]

            # ---- GroupNorm stats: tile 0 on DVE (bn_stats), tile 1 on the
            # scalar engine (Copy/Square + accum_out), in DMA-half granules ----
            with tc.tile_pool(name="gn", bufs=2) as gn, \
                 tc.tile_pool(name="gnps", bufs=1, space="PSUM") as gnps:
                mx_t = []
                # tile 0 (8 pieces) + first 2 pieces of tile 1: DVE bn_stats;
                # remaining 6 pieces of tile 1: scalar Copy/Square + accum_out.
                # Balances ~6.8us on each engine while the x8 DMA streams in.
                stats = gn.tile([P, 8, nc.vector.BN_STATS_DIM], F32, tag="st")
                for s in range(N // 512):
                    nc.vector.bn_stats(out=stats[:, s, :], in_=x8[:, 0, s * 512:(s + 1) * 512])
                statsb = gn.tile([P, 3, nc.vector.BN_STATS_DIM], F32, tag="stb")
                for s in range(3):
                    nc.vector.bn_stats(out=statsb[:, s, :], in_=x8[:, 1, s * 512:(s + 1) * 512])
                scr = gn.tile([P, NHALF], FP8, tag="scr")
                acc = gn.tile([P, 4], F32, tag="acc")  # sum_a|sum_b|sq_a|sq_b
                nc.scalar.activation(
                    out=scr[:, 0:512], in_=x8[:, 1, 1536:2048],
                    func=mybir.ActivationFunctionType.Copy, accum_out=acc[:, 0:1])
                nc.scalar.activation(
                    out=scr, in_=x8[:, 1, 2048:N],
                    func=mybir.ActivationFunctionType.Copy, accum_out=acc[:, 1:2])
                nc.scalar.activation(
                    out=scr[:, 0:512], in_=x8[:, 1, 1536:2048],
                    func=mybir.ActivationFunctionType.Square, accum_out=acc[:, 2:3])
                nc.scalar.activation(
                    out=scr, in_=x8[:, 1, 2048:N],
                    func=mybir.ActivationFunctionType.Square, accum_out=acc[:, 3:4])
                mv = gn.tile([P, nc.vector.BN_AGGR_DIM], F32, tag="mv")
                nc.vector.bn_aggr(out=mv, in_=stats)
                mx0 = gn.tile([P, 2], F32, tag="mx0")
                nc.vector.tensor_copy(out=mx0[:, 0:1], in_=mv[:, 0:1])
                msq = gn.tile([P, 1], F32, tag="msq")
                nc.vector.tensor_mul(out=msq, in0=mv[:, 0:1], in1=mv[:, 0:1])
                nc.vector.tensor_add(out=mx0[:, 1:2], in0=mv[:, 1:2], in1=msq)
                mx_t.append(mx0)
                # combine tile-1 DVE piece (1024 cols) with scalar pieces (3072)
                mvb = gn.tile([P, nc.vector.BN_AGGR_DIM], F32, tag="mvb")
                nc.vector.bn_aggr(out=mvb, in_=statsb)
                sumb = gn.tile([P, 2], F32, tag="sumb")  # sum | sumsq of DVE share
                nc.vector.tensor_scalar_mul(out=sumb[:, 0:1], in0=mvb[:, 0:1], scalar1=1536.0)
                msqb = gn.tile([P, 1], F32, tag="msqb")
                nc.vector.tensor_mul(out=msqb, in0=mvb[:, 0:1], in1=mvb[:, 0:1])
                nc.vector.tensor_add(out=msqb, in0=msqb, in1=mvb[:, 1:2])
                nc.vector.tensor_scalar_mul(out=sumb[:, 1:2], in0=msqb, scalar1=1536.0)
                mx1 = gn.tile([P, 2], F32, tag="mx1")
                ssum = gn.tile([P, 2], F32, tag="ssum")
                nc.vector.tensor_add(out=ssum[:, 0:1], in0=acc[:, 0:1], in1=acc[:, 1:2])
                nc.vector.tensor_add(out=ssum[:, 1:2], in0=acc[:, 2:3], in1=acc[:, 3:4])
                nc.vector.tensor_add(out=ssum, in0=ssum, in1=sumb)
                nc.vector.tensor_scalar_mul(out=mx1, in0=ssum, scalar1=1.0 / float(N))
                mx_t.append(mx1)

                for t in range(CT):
                    gps = gnps.tile([GT, 2], F32, tag="gps")
                    nc.tensor.matmul(gps, ind16_sb, mx_t[t], start=True, stop=True)
                    gsb = gn.tile([GT, 2], F32, tag="gsb")
                    nc.vector.tensor_copy(out=gsb, in_=gps)
                    mg2 = gn.tile([GT, 1], F32, tag="mg2")
                    nc.vector.tensor_mul(out=mg2, in0=gsb[:, 0:1], in1=gsb[:, 0:1])
                    varg = gn.tile([GT, 1], F32, tag="varg")
                    nc.vector.tensor_sub(out=varg, in0=gsb[:, 1:2], in1=mg2)
                    sd = gn.tile([GT, 1], F32, tag="sd")
                    nc.scalar.activation(
                        out=sd, in_=varg,
                        func=mybir.ActivationFunctionType.Sqrt,
                        bias=eps_sb, scale=1.0,
                    )
                    g2 = gn.tile([GT, 2], F32, tag="g2")
                    nc.vector.tensor_copy(out=g2[:, 0:1], in_=gsb[:, 0:1])
                    nc.vector.reciprocal(out=g2[:, 1:2], in_=sd)

                    bc = gnps.tile([P, 2], F32, tag="bc")
                    nc.tensor.matmul(bc, indb_sb, g2, start=True, stop=True)
                    nc.vector.tensor_mul(out=scale_sb[t], in0=gamma_sb[t], in1=bc[:, 1:2])
                    sh1 = gn.tile([P, 1], F32, tag="sh1")
                    nc.vector.tensor_mul(out=sh1, in0=bc[:, 0:1], in1=scale_sb[t])
                    nc.vector.tensor_sub(out=shift_sb[t], in0=beta_sb[t], in1=sh1)

                # ---- fp8 weights (scaled by 64 * GN scale) + effective biases ----
                wq8 = const.tile([P, CT, D], FP8, tag="wq8")
                wk8 = const.tile([P, CT, D], FP8, tag="wk8")
                wv8 = const.tile([P, CT, C], FP8, tag="wv8")
                wp8 = const.tile([P, CT, C], FP8, tag="wp8")
                s64 = [small.tile([P, 1], F32, tag=f"s64_{t}", name=f"s64_{t}") for t in range(CT)]
                for t in range(CT):
                    nc.vector.tensor_scalar_mul(out=s64[t], in0=scale_sb[t], scalar1=WS)
                    nc.vector.tensor_scalar_mul(out=wq8[:, t, :], in0=wqt_sb[t], scalar1=s64[t])
                    nc.vector.tensor_scalar_mul(out=wk8[:, t, :], in0=wkt_sb[t], scalar1=s64[t])
                    nc.vector.tensor_scalar_mul(out=wv8[:, t, :], in0=wvt_sb[t], scalar1=s64[t])
                    nc.vector.tensor_scalar_mul(out=wp8[:, t, :], in0=wpt_sb[t], scalar1=WS)

                bp_eff_t = []
                with tc.tile_pool(name="bps", bufs=1, space="PSUM") as bps:
                    bq64 = small.tile([D, 1], F32, tag="bq64")
                    bk64 = small.tile([D, 1], F32, tag="bk64")
                    psq = bps.tile([D, 1], F32, tag="pq")
                    psk = bps.tile([D, 1], F32, tag="pk")
                    for t in range(CT):
                        nc.tensor.matmul(psq, wqt_sb[t], shift_sb[t], start=(t == 0), stop=(t == CT - 1))
                        nc.tensor.matmul(psk, wkt_sb[t], shift_sb[t], start=(t == 0), stop=(t == CT - 1))
                    nc.vector.scalar_tensor_tensor(
                        out=bq64, in0=psq, scalar=1.0, in1=bq_sb, op0=MULT, op1=ADD)
                    nc.vector.tensor_scalar_mul(out=bq64, in0=bq64, scalar1=WS)
                    nc.vector.scalar_tensor_tensor(
                        out=bk64, in0=psk, scalar=1.0, in1=bk_sb, op0=MULT, op1=ADD)
                    nc.vector.tensor_scalar_mul(out=bk64, in0=bk64, scalar1=WS)

                    bv_eff = [small.tile([P, 1], F32, tag=f"bve{e}", name=f"bve{e}") for e in range(CT)]
                    for e in range(CT):
                        ps3 = bps.tile([P, 1], F32, tag=f"pv{e}", name=f"psv{e}")
                        for t in range(CT):
                            nc.tensor.matmul(
                                ps3, wvt_sb[t][:, e * P:(e + 1) * P], shift_sb[t],
                                start=(t == 0), stop=(t == CT - 1),
                            )
                        nc.vector.tensor_add(out=bv_eff[e], in0=ps3, in1=bv_sb[e])
                    for f in range(CT):
                        ps4 = bps.tile([P, 1], F32, tag=f"pp{f}", name=f"psp{f}")
                        for e in range(CT):
                            nc.tensor.matmul(
                                ps4, wpt_sb[e][:, f * P:(f + 1) * P], bv_eff[e],
                                start=(e == 0), stop=(e == CT - 1),
                            )
                        bp_eff = small.tile([P, 1], F32, tag=f"bpe{f}", name=f"bpe{f}")
                        nc.vector.tensor_add(out=bp_eff, in0=ps4, in1=bp_sb[f])
                        bp_eff_t.append(bp_eff)

            # ---- q/k projections + v^T pairs 0/1 (rest made inside chunk 0) ----
            q8 = const.tile([D, NQ], FP8, tag="q8")
            k8 = const.tile([D, N], FP8, tag="k8")
            vt8 = const.tile([P, NMB, C], FP8, tag="vt8")

            def make_v_pair(pool, pr):
                vp = pool.tile([P, 2, C], F32, tag="pj", name=f"vp{pr}")
                for mloc in range(2):
                    mb = pr * 2 + mloc
                    ms = slice(mb * MB, (mb + 1) * MB)
                    nc.tensor.matmul(vp[:, mloc, :], x8[:, :, ms], wv8,
                                     start=True, stop=True, perf_mode=DR)
                nc.vector.tensor_copy(out=vt8[:, pr * 2:(pr + 1) * 2, :], in_=vp)

            with tc.tile_pool(name="qkps", bufs=2, space="PSUM") as qkps, \
                 tc.tile_pool(name="vtps", bufs=2, space="PSUM") as vtps:
                def project(dst, bias, w8, ns):
                    pr_ = qkps.tile([D, CH], F32, tag="kp", name="prj")
                    nc.tensor.matmul(pr_, w8, x8[:, :, ns], start=True, stop=True,
                                     perf_mode=DR)
                    nc.scalar.activation(
                        out=dst, in_=pr_,
                        func=mybir.ActivationFunctionType.Identity,
                        bias=bias, scale=1.0,
                    )

                project(q8[:, 0:CH], bq64, wq8, slice(0, CH))
                for ck in range(N // CH):
                    ns = slice(ck * CH, (ck + 1) * CH)
                    project(k8[:, ns], bk64, wk8, ns)
                for cq in range(1, NQ // CH):
                    ns = slice(cq * CH, (cq + 1) * CH)
                    project(q8[:, ns], bq64, wq8, ns)
                make_v_pair(vtps, 0)
                make_v_pair(vtps, 1)

            # ---- attention ----
            ones_b = ones8[:, None, :].broadcast_to([P, CT, 1])
            with tc.tile_pool(name="stps", bufs=2, space="PSUM") as stps, \
                 tc.tile_pool(name="attps", bufs=1, space="PSUM") as attps, \
                 tc.tile_pool(name="rsps", bufs=1, space="PSUM") as rsps, \
                 tc.tile_pool(name="pjps", bufs=1, space="PSUM") as pjps, \
                 tc.tile_pool(name="pp", bufs=6) as pp, \
                 tc.tile_pool(name="attsb", bufs=2) as attsb, \
                 tc.tile_pool(name="osb", bufs=4) as osb, \
                 tc.tile_pool(name="rsb", bufs=2) as rsb:
                # residual base: DMA'd + biased here so the 2MB xq transfer
                # never gates the GN/QKV/attention critical path
                for t in range(CT):
                    cs = slice(t * P, (t + 1) * P)
                    xqt = const.tile([P, NQ], F32, tag=f"xqt{t}", name=f"xqt{t}")
                    (nc.gpsimd if t == 0 else nc.scalar).dma_start(out=xqt, in_=xq_ext[cs, :])
                    nc.vector.tensor_scalar_add(out=xqb[t], in0=xqt, scalar1=bp_eff_t[t])
                pend = None

                def epilogue_a(ep):
                    # DVE-only: att8 = att2 * 2^-6 = sum(P*v) (fp8 cast), and
                    # the denominator reciprocal + gpsimd broadcast. The
                    # 1/denominator is applied after the projection so this
                    # never gates the PE.
                    ns_p, att2_p, rs_p = ep
                    att8 = attsb.tile([P, CT, CH], FP8, tag="att8")
                    for i in range(CT):
                        nc.vector.tensor_scalar_mul(
                            out=att8[:, i, :], in0=att2_p[:, i, :], scalar1=1.0 / WS)
                    rec1 = rsb.tile([1, CH], F32, tag="rec1")
                    nc.vector.reciprocal_approx_fast(out=rec1, in_=rs_p)
                    rec = rsb.tile([P, CH], F32, tag="rec")
                    nc.gpsimd.partition_broadcast(rec, rec1, channels=P)
                    return att8, rec

                def epilogue_b(ep, att8, rec):
                    # PE projection + fused rescale/residual + output DMA,
                    # emitted two pairs later so att8 is ready when the
                    # in-order PE reaches the proj matmuls.
                    ns_p, att2_p, rs_p = ep
                    for f in range(CT):
                        fs = slice(f * P, (f + 1) * P)
                        pj = pjps.tile([P, CH], F32, tag="pj", name=f"pj{f}")
                        nc.tensor.matmul(pj, wp8[:, :, fs], att8,
                                         start=True, stop=True, perf_mode=DR)
                        t1 = osb.tile([P, CH], F32, tag="t1")
                        nc.vector.tensor_mul(out=t1, in0=pj, in1=rec)
                        o = osb.tile([P, CH], F32, tag="o")
                        nc.vector.scalar_tensor_tensor(
                            out=o, in0=t1, scalar=1.0 / WS,
                            in1=xqb[f][:, ns_p], op0=MULT, op1=ADD)
                        nc.sync.dma_start(out=out_ext[fs, ns_p], in_=o)

                for ch in range(NCH):
                    ns = slice(ch * CH, (ch + 1) * CH)
                    att2 = attps.tile([P, CT, CH], F32, tag="att2")
                    rs = rsps.tile([1, CH], F32, tag="rs")
                    p_tiles = [None] * NPR
                    for g in range(NPR + 1):
                        if g < NPR:
                            stg = stps.tile([P, CT, CH], F32, tag="stg")
                            for j in range(2):
                                mb = g * 2 + j
                                nc.tensor.matmul(stg[:, j, :],
                                                 k8[:, mb * MB:(mb + 1) * MB],
                                                 q8[:, ns],
                                                 start=True, stop=True)
                            pg = pp.tile([P, CT, CH], FP8, tag="pg")
                            nc.scalar.activation(
                                out=pg, in_=stg,
                                func=mybir.ActivationFunctionType.Exp,
                                scale=EXP_SCALE,
                            )
                            p_tiles[g] = pg
                        if g == 1 and pend is not None:
                            ep_state = epilogue_a(pend)
                        if g == 3 and pend is not None:
                            epilogue_b(pend, *ep_state)
                            pend = None
                        if g >= 1:
                            gp = g - 1
                            pg = p_tiles[gp]
                            nc.tensor.matmul(rs, ones_b, pg,
                                             start=(gp == 0), stop=(gp == NPR - 1),
                                             perf_mode=DR)
                            for e in range(CT):
                                nc.tensor.matmul(
                                    att2[:, e, :],
                                    vt8[:, 2 * gp:2 * gp + 2, e * P:(e + 1) * P],
                                    pg,
                                    start=(gp == 0), stop=(gp == NPR - 1),
                                    perf_mode=DR,
                                )
                            # chunk 0 doubles as the v^T production phase:
                            # pair gp+2 is built in the (otherwise idle) pj bank
                            if ch == 0 and gp + 2 < NPR:
                                make_v_pair(pjps, gp + 2)
                    pend = (ns, att2, rs)
                ep_state = epilogue_a(pend)
                epilogue_b(pend, *ep_state)

    nc.compile()
    _CACHE["nc"] = nc
    return nc


def _make_in_maps(x, gamma, beta, wq, bq, wk, bk, wv, bv, wp, bp):
    x = np.ascontiguousarray(np.asarray(x, dtype=np.float32))

    GT = G // CT
    ind16 = np.zeros((P, GT), np.float32)
    for c in range(P):
        ind16[c, c // GS] = 1.0 / GS
    indb = np.zeros((GT, P), np.float32)
    for c in range(P):
        indb[c // GS, c] = 1.0

    wall = np.concatenate(
        [
            np.asarray(wq, np.float32).T,
            np.asarray(wk, np.float32).T,
            np.asarray(wv, np.float32).T,
            np.asarray(wp, np.float32).T,
        ],
        axis=1,
    )
    bc4 = np.stack(
        [
            np.asarray(gamma, np.float32),
            np.asarray(beta, np.float32),
            np.asarray(bv, np.float32),
            np.asarray(bp, np.float32),
        ],
        axis=1,
    )
    bqk = np.stack([np.asarray(bq, np.float32), np.asarray(bk, np.float32)], axis=1)

    common = {
        "wall": np.ascontiguousarray(wall),
        "bc4": np.ascontiguousarray(bc4),
        "bqk": np.ascontiguousarray(bqk),
        "ind16": ind16,
        "indb": indb,
    }

    xf = x.reshape(B, C, N)
    # x8[p, t*N + n] = x[t*128+p, n] in fp8
    x8_all = np.ascontiguousarray(
        xf.reshape(B, CT, P, N).transpose(0, 2, 1, 3).reshape(B, P, CT * N)
    ).astype(ml_dtypes.float8_e4m3)
    in_maps = []
    for core in range(8):
        b, half = core // 2, core % 2
        m = dict(common)
        m["x8"] = x8_all[b]
        m["xq"] = np.ascontiguousarray(xf[b][:, half * NQ:(half + 1) * NQ])
        in_maps.append(m)
    return in_maps


def kernel(x, gamma, beta, wq, bq, wk, bk, wv, bv, wp, bp):
    nc = _build()
    in_maps = _make_in_maps(x, gamma, beta, wq, bq, wk, bk, wv, bv, wp, bp)
    global _last_in_maps
    _last_in_maps = in_maps
    res = run_bass_kernel_spmd(nc, in_maps, list(range(8)))

    y = np.empty((B, C, N), np.float32)
    for core in range(8):
        b, half = core // 2, core % 2
        y[b][:, half * NQ:(half + 1) * NQ] = res.results[core]["out"]
    return y.reshape(B, C, H, W)
